# revision 9
# baseline (speedup 1.0000x reference)
"""Trainium2 Bass kernel for ExpBertSelfAttention (B=2, S=2048, D=1024, H=16).

Sharding: 8 cores; core c handles batch b=c//4 and 4 consecutive heads
4*(c%4)..4*(c%4)+3 (data-parallel on B, tensor-parallel on heads).  The dense
output projection is row-parallel, so each core returns a partial [S, D] sum
in bf16; the host sums the 4 partials per batch and adds b_dense.

v4: f32r matmuls (self-loading weights -- bf16 stationaries cost a separate
125ns Ldweights SEQ instruction each, and fp8 quantization measures 2-3%
output error per tensor, over the 2e-2 gate).  bf16 only where it buys DVE
throughput: probs and the mask (both bf16 -> the in-place mask multiply runs
in the DVE 2x mode), so the PV and dense matmuls are bf16 (their Ldweights
cost is bounded).  Per-core structure:
  - qkvT [128, 4(Qp0,Qp1,Kp0,Kp1), S] f32r: head h at partitions
    64*(h%2)..+64 of pair tile h//2; 1/sqrt(64) folded into Wq host-side.
  - V feat-major tiles -> PE transpose (f32r) -> v_sb [128, h, kt, 65] bf16
    with a ones column at 64, so PV emits softmax row sums for free.
  - scores per (head, kt): two k=64 f32r matmuls into a [128, 1024] PSUM
    tile; exp on ACT writes bf16 probs; DVE multiplies the {0,1} bf16 mask
    in-place (2x mode, exact zeros).  Mask loaded once per q-chunk.
  - normalize: DVE copies the PSUM rowsum row to SBUF f32r, a k=1 f32r
    matmul broadcasts it over 64 partitions, DVE reciprocal + multiply
    (PSUM x SBUF -> bf16 ctx); odd heads staged via SBUF->SBUF DMA into
    partitions 64-127 of ctx_pair.
  - dense per q-chunk right after its 4 heads finish (overlaps the next
    chunk's attention on ACT/DVE): bf16 2-step k=128 accumulation, PSUM ->
    bf16 copies split across ACT and DVE, then DMA out.
"""

import os
import sys

for _p in ("/opt/trn_rl_repo", "/root/.axon_site/_ro/trn_rl_repo"):
    if os.path.isdir(_p) and _p not in sys.path:
        sys.path.insert(0, _p)

import numpy as np
import ml_dtypes

import concourse.bass as bass
import concourse.tile as tile
from concourse import bacc, mybir
from concourse import bass_utils

B, S, D, H = 2, 2048, 1024, 16
HD = D // H  # 64
SCALE = 8.0  # sqrt(HD)
NCORES = 8
HPC = H // (NCORES // B)  # heads per core = 4
P = 128
KT_S = S // P  # 16 key tiles

F32 = mybir.dt.float32
F32R = mybir.dt.float32r
BF16 = mybir.dt.bfloat16
AF = mybir.ActivationFunctionType
MUL = mybir.AluOpType.mult


def build_program():
    nc = bacc.Bacc("TRN2", target_bir_lowering=False, debug=False,
                   num_devices=NCORES)

    hsT = nc.dram_tensor("hsT", [D, S], F32R, kind="ExternalInput").ap()
    wqkv = nc.dram_tensor("wqkv", [D, 6 * P], F32R, kind="ExternalInput").ap()
    bqkv = nc.dram_tensor("bqkv", [P, 6], F32, kind="ExternalInput").ap()
    maskdr = nc.dram_tensor("maskdr", [S, S], BF16, kind="ExternalInput").ap()
    wd = nc.dram_tensor("wd", [2 * P, D], BF16, kind="ExternalInput").ap()
    y = nc.dram_tensor("y", [S, D], BF16, kind="ExternalOutput").ap()

    with tile.TileContext(nc) as tc:
        with tc.tile_pool(name="persist", bufs=1) as persist:
            qkvT = persist.tile([P, 4, S], F32R)           # 32 KB/part
            v_sb = persist.tile([P, HPC, KT_S, HD + 1], BF16)
            ctx_pair = persist.tile([P, 2, S], BF16)       # 8 KB/part
            wd_sb = persist.tile([P, 2, D], BF16)          # 4 KB/part
            bqk_sb = persist.tile([P, 6], F32)
            mask_sb = [persist.tile([P, KT_S, 1024], BF16,
                                    name=f"mask{qc}") for qc in range(2)]
            ones32 = persist.tile([1, HD], F32R)
            ident_f = persist.tile([P, P], F32R)

            from concourse.masks import make_identity
            ident_f32 = persist.tile([P, P], F32)
            make_identity(nc, ident_f32[:])
            nc.vector.tensor_copy(ident_f[:], ident_f32[:])
            o32f = persist.tile([1, HD], F32)
            nc.vector.memset(o32f[:], 1.0)
            nc.vector.tensor_copy(ones32[:], o32f[:])
            onesvf = persist.tile([P, KT_S], F32)
            nc.vector.memset(onesvf[:], 1.0)
            for h in range(HPC):
                nc.vector.tensor_copy(
                    v_sb[:, h, :, HD:HD + 1].rearrange("p k one -> p (k one)"),
                    onesvf[:])

            nc.sync.dma_start(wd_sb[:], wd.rearrange("(t p) n -> p t n", p=P))
            nc.sync.dma_start(bqk_sb[:], bqkv)
            # mask loads issued early; 4 kt-groups per q-chunk
            for qc in range(2):
                for g in range(4):
                    nc.sync.dma_start(
                        mask_sb[qc][:, 4 * g:4 * g + 4, :],
                        maskdr[g * 512:(g + 1) * 512,
                               qc * 1024:(qc + 1) * 1024].rearrange(
                                   "(kt p) q -> p kt q", p=P))

            # ---------------- Phase 1: QKV projection ----------------
            with (
                tc.tile_pool(name="p1sb", bufs=1) as p1sb,
                tc.tile_pool(name="hsp", bufs=2) as hsp,
                tc.tile_pool(name="qkps", bufs=6, space="PSUM") as qkps,
                tc.tile_pool(name="vtps", bufs=2, space="PSUM") as vtps,
            ):
                wqkv_sb = p1sb.tile([P, 8, 6 * P], F32R)     # 24 KB/part
                vfeat = p1sb.tile([P, 2, S], F32R)           # 16 KB/part
                hsT_r = hsT.rearrange("(t p) n -> p t n", p=P)
                nc.sync.dma_start(wqkv_sb[:],
                                  wqkv.rearrange("(t p) n -> p t n", p=P))
                hs_bufs = []
                for nch in range(4):
                    hb_t = hsp.tile([P, 8, 512], F32R, tag="hs")  # 16 KB
                    nc.sync.dma_start(hb_t[:],
                                      hsT_r[:, :, nch * 512:(nch + 1) * 512])
                    hs_bufs.append(hb_t)
                for nch in range(4):
                    ns = slice(nch * 512, (nch + 1) * 512)
                    hb_t = hs_bufs[nch]
                    ps_l = [qkps.tile([P, 512], F32, tag="qk",
                                      name=f"qk{nch}_{mt}")
                            for mt in range(6)]
                    for ktp in range(8):
                        for mt in range(6):
                            nc.tensor.matmul(
                                ps_l[mt][:],
                                wqkv_sb[:, ktp, mt * P:(mt + 1) * P],
                                hb_t[:, ktp, :],
                                start=(ktp == 0), stop=(ktp == 7))
                    for mt in range(4):
                        nc.vector.tensor_scalar_add(
                            qkvT[:, mt, ns], ps_l[mt][:],
                            bqk_sb[:, mt:mt + 1])
                    for pr in range(2):
                        nc.vector.tensor_scalar_add(
                            vfeat[:, pr, ns], ps_l[4 + pr][:],
                            bqk_sb[:, 4 + pr:5 + pr])
                    for kti in range(4):
                        kt = nch * 4 + kti
                        for pr in range(2):
                            tp = vtps.tile([P, P], F32R, tag="vt")
                            nc.tensor.transpose(
                                tp[:], vfeat[:, pr, kt * P:(kt + 1) * P],
                                ident_f[:])
                            nc.vector.tensor_copy(
                                v_sb[:, 2 * pr:2 * pr + 2, kt, 0:HD],
                                tp[:].rearrange("p (h j) -> p h j", h=2))

            # ---------------- Phase 2: attention ----------------
            with (
                tc.tile_pool(name="pp", bufs=5) as pp,
                tc.tile_pool(name="np_", bufs=2) as np_,
                tc.tile_pool(name="yp", bufs=4) as yp,
                tc.tile_pool(name="sps", bufs=2, space="PSUM") as sps,
                tc.tile_pool(name="cps", bufs=2, space="PSUM") as cps,
            ):
                for qc in range(2):
                    q0 = qc * 1024
                    for h in range(HPC):
                        hb = 64 * (h % 2)   # partition base of this head
                        pr = h // 2         # pair tile index
                        ctx = cps.tile([HD + 1, 1024], F32, tag="ctx")
                        for kt in range(KT_S):
                            s_ps = sps.tile([P, 1024], F32, tag="s")
                            for ch in range(2):
                                cs = slice(ch * 512, (ch + 1) * 512)
                                nc.tensor.matmul(
                                    s_ps[:, cs],
                                    qkvT[hb:hb + HD, 2 + pr,
                                         kt * P:(kt + 1) * P],
                                    qkvT[hb:hb + HD, 0 + pr,
                                         q0 + ch * 512:q0 + (ch + 1) * 512],
                                    start=True, stop=True)
                            prt = pp.tile([P, 1024], BF16, tag="probs")
                            nc.scalar.activation(prt[:], s_ps[:], AF.Exp)
                            nc.vector.tensor_tensor(
                                prt[:], prt[:], mask_sb[qc][:, kt, :], op=MUL)
                            for ch in range(2):
                                cs = slice(ch * 512, (ch + 1) * 512)
                                nc.tensor.matmul(
                                    ctx[:, cs],
                                    v_sb[:, h, kt, :],
                                    prt[:, cs],
                                    start=(kt == 0), stop=(kt == KT_S - 1))
                        # normalize: rowsum is PSUM row 64 (ones col of v_sb)
                        rrow = np_.tile([1, 1024], F32R, tag="rrow")
                        nc.vector.tensor_copy(rrow[:], ctx[HD:HD + 1, :])
                        rb = sps.tile([P, 1024], F32, tag="s",
                                      name=f"rb{qc}_{h}")
                        for ch in range(2):
                            cs = slice(ch * 512, (ch + 1) * 512)
                            nc.tensor.matmul(rb[0:HD, cs], ones32[:],
                                             rrow[:, cs],
                                             start=True, stop=True)
                        rbi = np_.tile([HD, 1024], F32, tag="rbi")
                        nc.vector.reciprocal_approx_fast(rbi[:], rb[0:HD, :])
                        if h % 2 == 0:
                            nc.vector.tensor_tensor(
                                ctx_pair[0:HD, pr, q0:q0 + 1024],
                                ctx[0:HD, :], rbi[:], op=MUL)
                        else:
                            stg = np_.tile([HD, 1024], BF16, tag="stg")
                            nc.vector.tensor_tensor(stg[:], ctx[0:HD, :],
                                                    rbi[:], op=MUL)
                            nc.sync.dma_start(
                                ctx_pair[HD:P, pr, q0:q0 + 1024], stg[:])
                    # dense for this q-chunk (overlaps next chunk's attention)
                    for mti in range(8):
                        mt = qc * 8 + mti
                        dp = sps.tile([P, 1024], F32, tag="s",
                                      name=f"d{qc}_{mti}")
                        for nch in range(2):
                            ncs = slice(nch * 512, (nch + 1) * 512)
                            for t in range(2):
                                nc.tensor.matmul(
                                    dp[:, ncs],
                                    ctx_pair[:, t, mt * P:(mt + 1) * P],
                                    wd_sb[:, t, ncs],
                                    start=(t == 0), stop=(t == 1))
                        ysb = yp.tile([P, D], BF16, tag="y")
                        nc.scalar.copy(ysb[:, 0:512], dp[:, 0:512])
                        nc.vector.tensor_copy(ysb[:, 512:1024],
                                              dp[:, 512:1024])
                        nc.sync.dma_start(y[mt * P:(mt + 1) * P, :], ysb[:])

    nc.compile()
    return nc


_NC = None


def get_program():
    global _NC
    if _NC is None:
        _NC = build_program()
    return _NC


def make_in_maps(hidden_states, attention_mask, W_qkv, b_qkv, W_dense,
                 b_dense):
    hs = np.asarray(hidden_states, np.float32)
    mask = np.asarray(attention_mask)
    W_qkv = np.asarray(W_qkv, np.float32)
    b_qkv = np.asarray(b_qkv, np.float32)
    W_dense = np.asarray(W_dense, np.float32)

    BFNP = ml_dtypes.bfloat16

    hsT = [np.ascontiguousarray(hs[b].T) for b in range(B)]
    maskT = [np.ascontiguousarray(
        np.where(mask[b, 0], 1.0, 0.0).astype(np.float32).T
    ).astype(BFNP) for b in range(B)]

    Wq, Wk, Wv = W_qkv[:, :D], W_qkv[:, D:2 * D], W_qkv[:, 2 * D:]
    bq, bk, bv = b_qkv[:D], b_qkv[D:2 * D], b_qkv[2 * D:]

    in_maps = []
    for c in range(NCORES):
        b = c // (NCORES // B)
        h0 = HPC * (c % (NCORES // B))
        cols0 = slice((h0 + 0) * HD, (h0 + 2) * HD)  # pair 0: heads 0,1
        cols1 = slice((h0 + 2) * HD, (h0 + 4) * HD)  # pair 1: heads 2,3
        # m-tiles [Qp0, Qp1, Kp0, Kp1, Vp0, Vp1]; 1/SCALE folded into Wq
        wqkv_c = np.ascontiguousarray(np.concatenate([
            Wq[:, cols0] / SCALE, Wq[:, cols1] / SCALE,
            Wk[:, cols0], Wk[:, cols1],
            Wv[:, cols0], Wv[:, cols1]], axis=1), dtype=np.float32)
        bqk_c = np.stack([
            bq[cols0] / SCALE, bq[cols1] / SCALE,
            bk[cols0], bk[cols1],
            bv[cols0], bv[cols1]], axis=1).astype(np.float32)
        wd_c = np.ascontiguousarray(
            W_dense[h0 * HD:(h0 + HPC) * HD, :]).astype(BFNP)
        in_maps.append({
            "hsT": hsT[b],
            "wqkv": wqkv_c,
            "bqkv": bqk_c,
            "maskdr": maskT[b],
            "wd": wd_c,
        })
    return in_maps


def kernel(hidden_states, attention_mask, W_qkv, b_qkv, W_dense, b_dense,
           **run_kwargs):
    nc = get_program()
    in_maps = make_in_maps(hidden_states, attention_mask, W_qkv, b_qkv,
                           W_dense, b_dense)
    res = bass_utils.run_bass_kernel_spmd(
        nc, in_maps, core_ids=list(range(NCORES)), **run_kwargs)
    out = np.zeros((B, S, D), np.float32)
    gpb = NCORES // B
    for c in range(NCORES):
        out[c // gpb] += res.results[c]["y"].astype(np.float32)
    out += np.asarray(b_dense, np.float32)
    if run_kwargs:
        kernel.last_results = res
    return out


# revision 10
# speedup vs baseline: 1.0530x; 1.0530x over previous
"""Trainium2 Bass kernel for ExpBertSelfAttention (B=2, S=2048, D=1024, H=16).

Sharding: 8 cores; core c handles batch b=c//4 and 4 consecutive heads
4*(c%4)..4*(c%4)+3 (data-parallel on B, tensor-parallel on heads).  The dense
output projection is row-parallel, so each core returns a partial [S, D] sum
in bf16; the host sums the 4 partials per batch and adds b_dense.

v4: f32r matmuls (self-loading weights -- bf16 stationaries cost a separate
125ns Ldweights SEQ instruction each, and fp8 quantization measures 2-3%
output error per tensor, over the 2e-2 gate).  bf16 only where it buys DVE
throughput: probs and the mask (both bf16 -> the in-place mask multiply runs
in the DVE 2x mode), so the PV and dense matmuls are bf16 (their Ldweights
cost is bounded).  Per-core structure:
  - qkvT [128, 4(Qp0,Qp1,Kp0,Kp1), S] f32r: head h at partitions
    64*(h%2)..+64 of pair tile h//2; 1/sqrt(64) folded into Wq host-side.
  - V feat-major tiles -> PE transpose (f32r) -> v_sb [128, h, kt, 65] bf16
    with a ones column at 64, so PV emits softmax row sums for free.
  - scores per (head, kt): two k=64 f32r matmuls into a [128, 1024] PSUM
    tile; exp on ACT writes bf16 probs; DVE multiplies the {0,1} bf16 mask
    in-place (2x mode, exact zeros).  Mask loaded once per q-chunk.
  - normalize: DVE copies the PSUM rowsum row to SBUF f32r, a k=1 f32r
    matmul broadcasts it over 64 partitions, DVE reciprocal + multiply
    (PSUM x SBUF -> bf16 ctx); odd heads staged via SBUF->SBUF DMA into
    partitions 64-127 of ctx_pair.
  - dense per q-chunk right after its 4 heads finish (overlaps the next
    chunk's attention on ACT/DVE): bf16 2-step k=128 accumulation, PSUM ->
    bf16 copies split across ACT and DVE, then DMA out.
"""

import os
import sys

for _p in ("/opt/trn_rl_repo", "/root/.axon_site/_ro/trn_rl_repo"):
    if os.path.isdir(_p) and _p not in sys.path:
        sys.path.insert(0, _p)

import numpy as np
import ml_dtypes

import concourse.bass as bass
import concourse.tile as tile
from concourse import bacc, mybir
from concourse import bass_utils

B, S, D, H = 2, 2048, 1024, 16
HD = D // H  # 64
SCALE = 8.0  # sqrt(HD)
NCORES = 8
HPC = H // (NCORES // B)  # heads per core = 4
P = 128
KT_S = S // P  # 16 key tiles

F32 = mybir.dt.float32
F32R = mybir.dt.float32r
BF16 = mybir.dt.bfloat16
AF = mybir.ActivationFunctionType
MUL = mybir.AluOpType.mult


def build_program():
    nc = bacc.Bacc("TRN2", target_bir_lowering=False, debug=False,
                   num_devices=NCORES)

    hsT = nc.dram_tensor("hsT", [D, S], F32R, kind="ExternalInput").ap()
    wqkv = nc.dram_tensor("wqkv", [D, 6 * P], F32R, kind="ExternalInput").ap()
    bqkv = nc.dram_tensor("bqkv", [P, 6], F32, kind="ExternalInput").ap()
    maskdr = nc.dram_tensor("maskdr", [S, S], BF16, kind="ExternalInput").ap()
    wd = nc.dram_tensor("wd", [2 * P, D], BF16, kind="ExternalInput").ap()
    y = nc.dram_tensor("y", [S, D], BF16, kind="ExternalOutput").ap()

    with tile.TileContext(nc) as tc:
        with tc.tile_pool(name="persist", bufs=1) as persist:
            qkvT = persist.tile([P, 4, S], F32R)           # 32 KB/part
            v_sb = persist.tile([P, HPC, KT_S, HD + 1], BF16)
            ctx_pair = persist.tile([P, 2, S], BF16)       # 8 KB/part
            wd_sb = persist.tile([P, 2, D], BF16)          # 4 KB/part
            bqk_sb = persist.tile([P, 6], F32)
            mask_sb = [persist.tile([P, KT_S, 1024], BF16,
                                    name=f"mask{qc}") for qc in range(2)]
            ones32 = persist.tile([1, HD], F32R)
            ident_f = persist.tile([P, P], F32R)

            from concourse.masks import make_identity
            ident_f32 = persist.tile([P, P], F32)
            make_identity(nc, ident_f32[:])
            nc.vector.tensor_copy(ident_f[:], ident_f32[:])
            o32f = persist.tile([1, HD], F32)
            nc.vector.memset(o32f[:], 1.0)
            nc.vector.tensor_copy(ones32[:], o32f[:])
            onesvf = persist.tile([P, KT_S], F32)
            nc.vector.memset(onesvf[:], 1.0)
            for h in range(HPC):
                nc.vector.tensor_copy(
                    v_sb[:, h, :, HD:HD + 1].rearrange("p k one -> p (k one)"),
                    onesvf[:])

            nc.sync.dma_start(wd_sb[:], wd.rearrange("(t p) n -> p t n", p=P))
            nc.sync.dma_start(bqk_sb[:], bqkv)
            # mask loads issued early; 4 kt-groups per q-chunk
            for qc in range(2):
                for g in range(4):
                    nc.sync.dma_start(
                        mask_sb[qc][:, 4 * g:4 * g + 4, :],
                        maskdr[g * 512:(g + 1) * 512,
                               qc * 1024:(qc + 1) * 1024].rearrange(
                                   "(kt p) q -> p kt q", p=P))

            # ---------------- Phase 1: QKV projection ----------------
            with (
                tc.tile_pool(name="p1sb", bufs=1) as p1sb,
                tc.tile_pool(name="hsp", bufs=2) as hsp,
                tc.tile_pool(name="qkps", bufs=6, space="PSUM") as qkps,
                tc.tile_pool(name="vtps", bufs=2, space="PSUM") as vtps,
            ):
                wqkv_sb = p1sb.tile([P, 8, 6 * P], F32R)     # 24 KB/part
                vfeat = p1sb.tile([P, 2, S], F32R)           # 16 KB/part
                hsT_r = hsT.rearrange("(t p) n -> p t n", p=P)
                nc.sync.dma_start(wqkv_sb[:],
                                  wqkv.rearrange("(t p) n -> p t n", p=P))
                hs_bufs = []
                for nch in range(4):
                    hb_t = hsp.tile([P, 8, 512], F32R, tag="hs")  # 16 KB
                    nc.sync.dma_start(hb_t[:],
                                      hsT_r[:, :, nch * 512:(nch + 1) * 512])
                    hs_bufs.append(hb_t)
                for nch in range(4):
                    ns = slice(nch * 512, (nch + 1) * 512)
                    hb_t = hs_bufs[nch]
                    ps_l = [qkps.tile([P, 512], F32, tag="qk",
                                      name=f"qk{nch}_{mt}")
                            for mt in range(6)]
                    for ktp in range(8):
                        for mt in range(6):
                            nc.tensor.matmul(
                                ps_l[mt][:],
                                wqkv_sb[:, ktp, mt * P:(mt + 1) * P],
                                hb_t[:, ktp, :],
                                start=(ktp == 0), stop=(ktp == 7))
                    for mt in range(4):
                        nc.vector.tensor_scalar_add(
                            qkvT[:, mt, ns], ps_l[mt][:],
                            bqk_sb[:, mt:mt + 1])
                    for pr in range(2):
                        nc.vector.tensor_scalar_add(
                            vfeat[:, pr, ns], ps_l[4 + pr][:],
                            bqk_sb[:, 4 + pr:5 + pr])
                    for kti in range(4):
                        kt = nch * 4 + kti
                        for pr in range(2):
                            tp = vtps.tile([P, P], F32R, tag="vt")
                            nc.tensor.transpose(
                                tp[:], vfeat[:, pr, kt * P:(kt + 1) * P],
                                ident_f[:])
                            nc.vector.tensor_copy(
                                v_sb[:, 2 * pr:2 * pr + 2, kt, 0:HD],
                                tp[:].rearrange("p (h j) -> p h j", h=2))

            # ---------------- Phase 2: attention ----------------
            with (
                tc.tile_pool(name="pp", bufs=5) as pp,
                tc.tile_pool(name="np_", bufs=2) as np_,
                tc.tile_pool(name="yp", bufs=4) as yp,
                tc.tile_pool(name="sps", bufs=2, space="PSUM") as sps,
                tc.tile_pool(name="cps", bufs=2, space="PSUM") as cps,
            ):
                def normalize(ctx, h, pr, q0):
                    # rowsum is PSUM row 64 (ones column of v_sb); broadcast
                    # on the otherwise-idle Pool engine, reciprocal+multiply
                    # on DVE.
                    rrow = np_.tile([1, 1024], F32R, tag="rrow")
                    nc.vector.tensor_copy(rrow[:], ctx[HD:HD + 1, :])
                    rbc = np_.tile([HD, 1024], F32, tag="rbc")
                    nc.gpsimd.partition_broadcast(rbc[:], rrow[:].bitcast(F32))
                    rbi = np_.tile([HD, 1024], F32, tag="rbi")
                    nc.vector.reciprocal_approx_fast(rbi[:], rbc[:])
                    if h % 2 == 0:
                        nc.vector.tensor_tensor(
                            ctx_pair[0:HD, pr, q0:q0 + 1024],
                            ctx[0:HD, :], rbi[:], op=MUL)
                    else:
                        stg = np_.tile([HD, 1024], BF16, tag="stg")
                        nc.vector.tensor_tensor(stg[:], ctx[0:HD, :],
                                                rbi[:], op=MUL)
                        nc.sync.dma_start(
                            ctx_pair[HD:P, pr, q0:q0 + 1024], stg[:])

                LAG = 2  # PV trails QK by this many kt tiles
                pending = None  # (ctx, h, pr, q0) awaiting normalize
                for qc in range(2):
                    q0 = qc * 1024
                    for h in range(HPC):
                        hb = 64 * (h % 2)   # partition base of this head
                        pr = h // 2         # pair tile index
                        ctx = cps.tile([HD + 1, 1024], F32, tag="ctx")

                        def emit_pv(kt, prt):
                            for ch in range(2):
                                cs = slice(ch * 512, (ch + 1) * 512)
                                nc.tensor.matmul(
                                    ctx[:, cs],
                                    v_sb[:, h, kt, :],
                                    prt[:, cs],
                                    start=(kt == 0), stop=(kt == KT_S - 1))

                        prts = {}
                        for kt in range(KT_S):
                            s_ps = sps.tile([P, 1024], F32, tag="s")
                            for ch in range(2):
                                cs = slice(ch * 512, (ch + 1) * 512)
                                nc.tensor.matmul(
                                    s_ps[:, cs],
                                    qkvT[hb:hb + HD, 2 + pr,
                                         kt * P:(kt + 1) * P],
                                    qkvT[hb:hb + HD, 0 + pr,
                                         q0 + ch * 512:q0 + (ch + 1) * 512],
                                    start=True, stop=True)
                            prt = pp.tile([P, 1024], BF16, tag="probs")
                            nc.scalar.activation(prt[:], s_ps[:], AF.Exp)
                            nc.vector.tensor_tensor(
                                prt[:], prt[:], mask_sb[qc][:, kt, :], op=MUL)
                            prts[kt] = prt
                            if kt >= LAG:
                                emit_pv(kt - LAG, prts.pop(kt - LAG))
                            if kt == LAG and pending is not None:
                                normalize(*pending)
                                pending = None
                        for kt in range(KT_S - LAG, KT_S):
                            emit_pv(kt, prts.pop(kt))
                        pending = (ctx, h, pr, q0)
                    # last head's normalize must precede this chunk's dense
                    normalize(*pending)
                    pending = None
                    # dense for this q-chunk
                    for mti in range(8):
                        mt = qc * 8 + mti
                        dp = sps.tile([P, 1024], F32, tag="s",
                                      name=f"d{qc}_{mti}")
                        for nch in range(2):
                            ncs = slice(nch * 512, (nch + 1) * 512)
                            for t in range(2):
                                nc.tensor.matmul(
                                    dp[:, ncs],
                                    ctx_pair[:, t, mt * P:(mt + 1) * P],
                                    wd_sb[:, t, ncs],
                                    start=(t == 0), stop=(t == 1))
                        ysb = yp.tile([P, D], BF16, tag="y")
                        nc.scalar.copy(ysb[:, 0:512], dp[:, 0:512])
                        nc.vector.tensor_copy(ysb[:, 512:1024],
                                              dp[:, 512:1024])
                        nc.sync.dma_start(y[mt * P:(mt + 1) * P, :], ysb[:])

    nc.compile()
    return nc


_NC = None


def get_program():
    global _NC
    if _NC is None:
        _NC = build_program()
    return _NC


def make_in_maps(hidden_states, attention_mask, W_qkv, b_qkv, W_dense,
                 b_dense):
    hs = np.asarray(hidden_states, np.float32)
    mask = np.asarray(attention_mask)
    W_qkv = np.asarray(W_qkv, np.float32)
    b_qkv = np.asarray(b_qkv, np.float32)
    W_dense = np.asarray(W_dense, np.float32)

    BFNP = ml_dtypes.bfloat16

    hsT = [np.ascontiguousarray(hs[b].T) for b in range(B)]
    maskT = [np.ascontiguousarray(
        np.where(mask[b, 0], 1.0, 0.0).astype(np.float32).T
    ).astype(BFNP) for b in range(B)]

    Wq, Wk, Wv = W_qkv[:, :D], W_qkv[:, D:2 * D], W_qkv[:, 2 * D:]
    bq, bk, bv = b_qkv[:D], b_qkv[D:2 * D], b_qkv[2 * D:]

    in_maps = []
    for c in range(NCORES):
        b = c // (NCORES // B)
        h0 = HPC * (c % (NCORES // B))
        cols0 = slice((h0 + 0) * HD, (h0 + 2) * HD)  # pair 0: heads 0,1
        cols1 = slice((h0 + 2) * HD, (h0 + 4) * HD)  # pair 1: heads 2,3
        # m-tiles [Qp0, Qp1, Kp0, Kp1, Vp0, Vp1]; 1/SCALE folded into Wq
        wqkv_c = np.ascontiguousarray(np.concatenate([
            Wq[:, cols0] / SCALE, Wq[:, cols1] / SCALE,
            Wk[:, cols0], Wk[:, cols1],
            Wv[:, cols0], Wv[:, cols1]], axis=1), dtype=np.float32)
        bqk_c = np.stack([
            bq[cols0] / SCALE, bq[cols1] / SCALE,
            bk[cols0], bk[cols1],
            bv[cols0], bv[cols1]], axis=1).astype(np.float32)
        wd_c = np.ascontiguousarray(
            W_dense[h0 * HD:(h0 + HPC) * HD, :]).astype(BFNP)
        in_maps.append({
            "hsT": hsT[b],
            "wqkv": wqkv_c,
            "bqkv": bqk_c,
            "maskdr": maskT[b],
            "wd": wd_c,
        })
    return in_maps


def kernel(hidden_states, attention_mask, W_qkv, b_qkv, W_dense, b_dense,
           **run_kwargs):
    nc = get_program()
    in_maps = make_in_maps(hidden_states, attention_mask, W_qkv, b_qkv,
                           W_dense, b_dense)
    res = bass_utils.run_bass_kernel_spmd(
        nc, in_maps, core_ids=list(range(NCORES)), **run_kwargs)
    out = np.zeros((B, S, D), np.float32)
    gpb = NCORES // B
    for c in range(NCORES):
        out[c // gpb] += res.results[c]["y"].astype(np.float32)
    out += np.asarray(b_dense, np.float32)
    if run_kwargs:
        kernel.last_results = res
    return out
